# revision 27
# baseline (speedup 1.0000x reference)
"""Trainium2 Bass kernel for a GQA causal attention block (B=2, S=2048,
HID=2048, 16 q-heads / 4 kv-heads, RoPE, causal softmax, output proj).

Sharding: core c in [0,8) handles batch b = c//4 and head-group g = c%4
(q-heads 4g..4g+3, kv-head g).  Wq/Wk/Wv are column-sharded by head group,
Wo row-sharded; each core emits a partial output and the host sums the 4
partials per batch.

Per-core kernel (all matmuls free-dim 512 where possible, bf16 inputs with
fp32 PSUM accumulation):
  - qT/kT computed in [d, s] layout directly (weights pre-transposed on
    host); RoPE applied in rotate-half form (weight rows pre-permuted
    evens-then-odds on host) via DVE ops on [64, 512] tiles.
  - scores computed TRANSPOSED, sT[k, q] = kT.T-tile @ qT, so the PV matmul
    consumes exp(sT) directly with no on-chip transposes.
  - softmax without max subtraction (scores ~N(0, 0.8); exp is safe in f32),
    denominator accumulated in f32 SBUF and reduced with a ones-matmul,
    normalization broadcast via a K=1 matmul + DVE multiply.
"""

import numpy as np
import ml_dtypes

try:
    import concourse  # noqa: F401
except ImportError:  # pragma: no cover - path fallback
    import sys

    for _p in ("/root/.axon_site/_ro/trn_rl_repo", "/opt/trn_rl_repo"):
        if _p not in sys.path:
            sys.path.append(_p)

from contextlib import ExitStack

import concourse.bass as bass
import concourse.tile as tile
from concourse import bacc, mybir
from concourse.bass_utils import run_bass_kernel_spmd

F32 = mybir.dt.float32
BF16 = mybir.dt.bfloat16

B = 2
S = 2048
HID = 2048
HEADS = 16
KV_HEADS = 4
HD = 128
HALF = HD // 2
QH = HEADS // KV_HEADS  # q heads per core (4)
LO = QH * HD  # local q/o width (512)
N_CORES = 8

NEG = -1.0e5  # additive causal mask value (exp -> exactly 0 in f32)


def _emit(ctx: ExitStack, tc: "tile.TileContext", aps: dict, s_len: int):
    nc = tc.nc
    IT = HID // 128  # contraction tiles (16)
    SC = s_len // 512  # s-chunks of 512
    KBT = s_len // 128  # 128-wide k blocks
    QBT = s_len // 512  # 512-wide q blocks

    xT, wqT, wkT, wvT, woT = aps["xT"], aps["wqT"], aps["wkT"], aps["wvT"], aps["woT"]
    cosq, sinq, cosk, sink = aps["cosq"], aps["sinq"], aps["cosk"], aps["sink"]
    mtri, outp = aps["mtri"], aps["outp"]

    # ---- pools ----
    xpool = ctx.enter_context(tc.tile_pool(name="xpool", bufs=2))
    spsum = ctx.enter_context(tc.tile_pool(name="spsum", bufs=4, space="PSUM"))
    ypsum = ctx.enter_context(tc.tile_pool(name="ypsum", bufs=2, space="PSUM"))
    lpsum = ctx.enter_context(tc.tile_pool(name="lpsum", bufs=2, space="PSUM"))
    ptpool = ctx.enter_context(tc.tile_pool(name="ptpool", bufs=4))
    ropet = ctx.enter_context(tc.tile_pool(name="ropet", bufs=4))
    bcpool = ctx.enter_context(tc.tile_pool(name="bcpool", bufs=2))
    invpool = ctx.enter_context(tc.tile_pool(name="invpool", bufs=2))
    outpool = ctx.enter_context(tc.tile_pool(name="outpool", bufs=3))

    # ---- persistent SBUF tensors ----
    def single(shape, dtype, name):
        t, free = tc.tile(shape, dtype, name=name)
        ctx.callback(free)
        return t

    wq_sb = single([128, IT, LO], BF16, "wq_sb")
    wk_sb = single([128, IT, HD], BF16, "wk_sb")
    wv_sb = single([128, IT, HD], BF16, "wv_sb")
    wo_sb = single([128, QH, HID], BF16, "wo_sb")
    cq_sb = single([128, s_len], F32, "cq_sb")  # [cos; cos] (q-scaled)
    sq_sb = single([128, s_len], F32, "sq_sb")  # [-sin; sin] (q-scaled)
    ck_sb = single([128, s_len], F32, "ck_sb")
    sk_sb = single([128, s_len], F32, "sk_sb")
    mtri_sb = single([128, 128], F32, "mtri_sb")
    qT_sb = single([128, QH, s_len], BF16, "qT_sb")
    kT_sb = single([128, s_len], BF16, "kT_sb")
    v_sb = single([128, KBT, HD], BF16, "v_sb")
    yT_sb = single([128, QH, s_len], BF16, "yT_sb")
    ones_col = single([128, 1], BF16, "ones_col")

    nc.vector.memset(ones_col, 1.0)

    # Weights on the sync DMA queue, x chunks on the gpsimd queue (parallel
    # rings) so the first matmuls start ~6us in.  wo is deferred until after
    # phase 1 so it doesn't delay startup.
    # Spread the startup loads over four independent DMA rings so the first
    # matmuls start as soon as possible: weights on sync, tables on scalar,
    # x chunks on gpsimd, wk/wv on vector.
    wq_r = wqT.rearrange("(it p) o -> p it o", p=128)
    for i4 in range(IT // 4):
        nc.sync.dma_start(
            out=wq_sb[:, i4 * 4 : (i4 + 1) * 4, :], in_=wq_r[:, i4 * 4 : (i4 + 1) * 4]
        )
    nc.scalar.dma_start(out=cq_sb, in_=cosq)
    nc.scalar.dma_start(out=sq_sb, in_=sinq)
    nc.scalar.dma_start(out=ck_sb, in_=cosk)
    nc.scalar.dma_start(out=sk_sb, in_=sink)
    nc.scalar.dma_start(out=mtri_sb, in_=mtri)
    nc.scalar.dma_start(out=wk_sb, in_=wkT.rearrange("(it p) o -> p it o", p=128))
    nc.scalar.dma_start(out=wv_sb, in_=wvT.rearrange("(it p) o -> p it o", p=128))

    xT_r = xT.rearrange("(it p) s -> p it s", p=128)

    def rope(ps, out_full, cos_sb, sin_sb, sc):
        """out = RoPE(ps) in rotate-half layout; ps is a [128, 512] psum tile
        whose partitions are [evens(64); odds(64)] of one head.
        out = ps * C2 + swap_halves(ps) * S2, C2 = [cos;cos], S2 = [-sin;sin].
        """
        cs = cos_sb[:, sc * 512 : (sc + 1) * 512]
        sn = sin_sb[:, sc * 512 : (sc + 1) * 512]
        m1 = ropet.tile([128, 512], F32, tag="m1", name="m1")
        m2 = ropet.tile([128, 512], F32, tag="m2", name="m2")
        nc.vector.tensor_mul(m1, ps, cs)
        nc.vector.tensor_mul(m2[0:HALF, :], ps[HALF:128, :], sn[0:HALF, :])
        nc.vector.tensor_mul(m2[HALF:128, :], ps[0:HALF, :], sn[HALF:128, :])
        nc.vector.tensor_add(out_full, m1, m2)

    # ---------------- phase 1: Q/K/V projections + RoPE ----------------
    for sc in range(SC):
        xs = xpool.tile([128, IT, 512], BF16, tag="xs", name="xs")
        for i4 in range(IT // 4):
            nc.gpsimd.dma_start(
                out=xs[:, i4 * 4 : (i4 + 1) * 4, :],
                in_=xT_r[:, i4 * 4 : (i4 + 1) * 4, sc * 512 : (sc + 1) * 512],
            )

        sl = slice(sc * 512, (sc + 1) * 512)
        # Q: per head, accumulate over i-tiles -> [128(d), 512(s)] psum
        for h in range(QH):
            ps_q = spsum.tile([128, 512], F32, tag="ps", name="ps_q")
            for it in range(IT):
                nc.tensor.matmul(
                    ps_q,
                    wq_sb[:, it, h * HD : (h + 1) * HD],
                    xs[:, it, :],
                    start=(it == 0),
                    stop=(it == IT - 1),
                )
            rope(ps_q, qT_sb[:, h, sl], cq_sb, sq_sb, sc)

        # K: one kv head
        ps_k = spsum.tile([128, 512], F32, tag="ps", name="ps_k")
        for it in range(IT):
            nc.tensor.matmul(
                ps_k,
                wk_sb[:, it, :],
                xs[:, it, :],
                start=(it == 0),
                stop=(it == IT - 1),
            )
        rope(ps_k, kT_sb[:, sl], ck_sb, sk_sb, sc)

        # V: natural [s, d] layout, four 128-row s-tiles per chunk
        for sj in range(4):
            st = sc * 4 + sj
            ps_v = spsum.tile([128, 512], F32, tag="ps", name="ps_v")
            for it in range(IT):
                nc.tensor.matmul(
                    ps_v[:, 0:HD],
                    xs[:, it, sj * 128 : (sj + 1) * 128],
                    wv_sb[:, it, :],
                    start=(it == 0),
                    stop=(it == IT - 1),
                )
            nc.scalar.copy(v_sb[:, st, :], ps_v[:, 0:HD])

    # prefetch wo during the attention phase
    nc.sync.dma_start(out=wo_sb, in_=woT.rearrange("(jt p) o -> p jt o", p=128))

    # ---------------- phase 2: attention ----------------
    # Software pipelining: emit the scores matmul for block kb, and the rest
    # of the chain (mask/exp/accumulate/PV) for block kb-LOOKAHEAD, so the PE
    # isn't stalled on the ACT exp of the immediately preceding block.
    LOOKAHEAD = 3
    pending = []  # deferred per-(h,J) finalizers

    def flush_pending():
        while pending:
            pending.pop(0)()

    for h in range(QH):
        for J in range(QBT):
            nkb = 4 * J + 4
            qsl = slice(J * 512, (J + 1) * 512)

            ps_y = ypsum.tile([128, 512], F32, tag="ps_y", name="ps_y")
            ps_l = lpsum.tile([1, 512], F32, tag="ps_l", name="ps_l")

            rest_q = []

            def emit_rest(kb, ps_s, h=h, J=J, ps_y=ps_y, ps_l=ps_l):
                nkb_ = 4 * J + 4
                r = kb - 4 * J
                lo = r * 128 if r >= 0 else 0
                if r >= 0:
                    nc.vector.tensor_add(
                        ps_s[:, lo : lo + 128], ps_s[:, lo : lo + 128], mtri_sb
                    )
                pt = ptpool.tile([128, 512], BF16, tag="pt", name="pt")
                nc.scalar.activation(
                    pt[:, lo:512], ps_s[:, lo:512], mybir.ActivationFunctionType.Exp
                )
                # softmax denominator: accumulate column sums of pt on the PE
                nc.tensor.matmul(
                    ps_l[:, lo:512],
                    ones_col,
                    pt[:, lo:512],
                    start=(kb == 0),
                    stop=(kb == nkb_ - 1),
                )
                nc.tensor.matmul(
                    ps_y[:, lo:512],
                    v_sb[:, kb, :],
                    pt[:, lo:512],
                    start=(kb == 0),
                    stop=(kb == nkb_ - 1),
                )

            for kb in range(nkb):
                r = kb - 4 * J  # >=0 on diagonal blocks
                lo = r * 128 if r >= 0 else 0

                ps_s = spsum.tile([128, 512], F32, tag="ps", name="ps_s")
                nc.tensor.matmul(
                    ps_s[:, lo:512],
                    kT_sb[:, kb * 128 : (kb + 1) * 128],
                    qT_sb[:, h, J * 512 + lo : (J + 1) * 512],
                    start=True,
                    stop=True,
                )
                rest_q.append((kb, ps_s))
                if kb == 1:
                    flush_pending()
                if len(rest_q) > LOOKAHEAD:
                    emit_rest(*rest_q.pop(0))
            while rest_q:
                emit_rest(*rest_q.pop(0))

            def finalize(h=h, J=J, ps_y=ps_y, ps_l=ps_l, qsl=qsl):
                # free both PSUM tiles immediately (unnormalized y to SBUF,
                # denominator to SBUF); normalize in place off the PE path.
                # Copies run on DVE so the ACT queue stays pure-exp.
                nc.vector.tensor_copy(yT_sb[:, h, qsl], ps_y)
                l_sb = invpool.tile([1, 512], F32, tag="l_sb", name="l_sb")
                nc.vector.tensor_copy(l_sb, ps_l)
                # broadcast l across partitions (GPSIMD), then 1/l on DVE
                lbc = bcpool.tile([128, 512], F32, tag="lbc", name="lbc")
                nc.gpsimd.partition_broadcast(lbc, l_sb)
                rinv = invpool.tile([128, 512], F32, tag="rinv", name="rinv")
                nc.vector.reciprocal_approx_fast(rinv, lbc)
                nc.vector.tensor_mul(yT_sb[:, h, qsl], yT_sb[:, h, qsl], rinv)

            pending.append(finalize)

    flush_pending()

    # ---------------- phase 3: output projection ----------------
    for st in range(KBT):
        for ob in range(HID // 512):
            ps_o = ypsum.tile([128, 512], F32, tag="ps_y", name="ps_o")
            for h in range(QH):
                nc.tensor.matmul(
                    ps_o,
                    yT_sb[:, h, st * 128 : (st + 1) * 128],
                    wo_sb[:, h, ob * 512 : (ob + 1) * 512],
                    start=(h == 0),
                    stop=(h == QH - 1),
                )
            o_sb = outpool.tile([128, 512], F32, tag="o_sb", name="o_sb")
            if (st * (HID // 512) + ob) % 2 == 0:
                nc.scalar.copy(o_sb, ps_o)
            else:
                nc.vector.tensor_copy(o_sb, ps_o)
            nc.sync.dma_start(
                out=outp[st * 128 : (st + 1) * 128, ob * 512 : (ob + 1) * 512],
                in_=o_sb,
            )


def build_module(s_len: int = S):
    nc = bacc.Bacc(
        "TRN2", target_bir_lowering=False, debug=False, enable_asserts=False
    )
    aps = {}
    aps["xT"] = nc.dram_tensor("xT", [HID, s_len], BF16, kind="ExternalInput").ap()
    aps["wqT"] = nc.dram_tensor("wqT", [HID, LO], BF16, kind="ExternalInput").ap()
    aps["wkT"] = nc.dram_tensor("wkT", [HID, HD], BF16, kind="ExternalInput").ap()
    aps["wvT"] = nc.dram_tensor("wvT", [HID, HD], BF16, kind="ExternalInput").ap()
    aps["woT"] = nc.dram_tensor("woT", [LO, HID], BF16, kind="ExternalInput").ap()
    aps["cosq"] = nc.dram_tensor("cosq", [128, s_len], F32, kind="ExternalInput").ap()
    aps["sinq"] = nc.dram_tensor("sinq", [128, s_len], F32, kind="ExternalInput").ap()
    aps["cosk"] = nc.dram_tensor("cosk", [128, s_len], F32, kind="ExternalInput").ap()
    aps["sink"] = nc.dram_tensor("sink", [128, s_len], F32, kind="ExternalInput").ap()
    aps["mtri"] = nc.dram_tensor("mtri", [128, 128], F32, kind="ExternalInput").ap()
    aps["outp"] = nc.dram_tensor("outp", [s_len, HID], F32, kind="ExternalOutput").ap()

    with tile.TileContext(nc) as tc:
        with ExitStack() as ctx:
            _emit(ctx, tc, aps, s_len)
    nc.compile()
    return nc


_MODULE_CACHE: dict = {}


def _get_module(s_len: int = S):
    if s_len not in _MODULE_CACHE:
        _MODULE_CACHE[s_len] = build_module(s_len)
    return _MODULE_CACHE[s_len]


_PERM = np.concatenate([np.arange(0, HD, 2), np.arange(1, HD, 2)])  # evens|odds


def make_in_maps(x, cos, sin, Wq, Wk, Wv, Wo, s_len: int = S):
    """Build the 8 per-core input maps (host-side sharding + layout prep)."""
    x = np.asarray(x, dtype=np.float32)
    cos = np.asarray(cos, dtype=np.float32)
    sin = np.asarray(sin, dtype=np.float32)
    Wq = np.asarray(Wq, dtype=np.float32)
    Wk = np.asarray(Wk, dtype=np.float32)
    Wv = np.asarray(Wv, dtype=np.float32)
    Wo = np.asarray(Wo, dtype=np.float32)

    bf = ml_dtypes.bfloat16
    scale = 1.0 / np.sqrt(HD)

    cosT = np.ascontiguousarray(cos.T)  # [64, S]
    sinT = np.ascontiguousarray(sin.T)
    # rotate-half tables: out = ps*[c;c] + swap_halves(ps)*[-s;s]
    cos2 = np.vstack([cosT, cosT])  # [128, S]
    sin2 = np.vstack([-sinT, sinT])
    cosq = (cos2 * scale).astype(np.float32)
    sinq = (sin2 * scale).astype(np.float32)
    cosk = cos2.astype(np.float32)
    sink = sin2.astype(np.float32)

    kk, qq = np.meshgrid(np.arange(128), np.arange(128), indexing="ij")
    mtri = np.where(kk <= qq, 0.0, NEG).astype(np.float32)

    Wq4 = Wq.reshape(HEADS, HD, HID)
    Wk4 = Wk.reshape(KV_HEADS, HD, HID)
    Wv4 = Wv.reshape(KV_HEADS, HD, HID)

    in_maps = []
    for c in range(N_CORES):
        b, g = divmod(c, KV_HEADS)
        hs = [g * QH + i for i in range(QH)]
        wq_l = Wq4[hs][:, _PERM, :].reshape(LO, HID)  # [512, 2048]
        wk_l = Wk4[g][_PERM, :]  # [128, 2048]
        wv_l = Wv4[g]  # [128, 2048]
        jcols = np.concatenate([np.arange(h * HD, (h + 1) * HD) for h in hs])
        wo_l = Wo[:, jcols]  # [2048, 512]

        in_maps.append(
            {
                "xT": np.ascontiguousarray(x[b].T).astype(bf),
                "wqT": np.ascontiguousarray(wq_l.T).astype(bf),
                "wkT": np.ascontiguousarray(wk_l.T).astype(bf),
                "wvT": np.ascontiguousarray(wv_l.T).astype(bf),
                "woT": np.ascontiguousarray(wo_l.T).astype(bf),
                "cosq": cosq,
                "sinq": sinq,
                "cosk": cosk,
                "sink": sink,
                "mtri": mtri,
            }
        )
    return in_maps


def combine_outputs(results):
    out = np.zeros((B, S, HID), dtype=np.float32)
    for c in range(N_CORES):
        b = c // KV_HEADS
        out[b] += results[c]["outp"]
    return out


def kernel(x, cos, sin, Wq, Wk, Wv, Wo):
    nc = _get_module(S)
    in_maps = make_in_maps(x, cos, sin, Wq, Wk, Wv, Wo, S)
    res = run_bass_kernel_spmd(nc, in_maps, core_ids=list(range(N_CORES)))
    return combine_outputs(res.results)


def run_traced(x, cos, sin, Wq, Wk, Wv, Wo, **trace_kwargs):
    """Like kernel() but with NTFF tracing; returns (output, BassKernelResults)."""
    nc = _get_module(S)
    in_maps = make_in_maps(x, cos, sin, Wq, Wk, Wv, Wo, S)
    res = run_bass_kernel_spmd(
        nc, in_maps, core_ids=list(range(N_CORES)), trace=True, **trace_kwargs
    )
    return combine_outputs(res.results), res


# revision 38
# speedup vs baseline: 1.0602x; 1.0602x over previous
"""Trainium2 Bass kernel for a GQA causal attention block (B=2, S=2048,
HID=2048, 16 q-heads / 4 kv-heads, RoPE, causal softmax, output proj).

Sharding: core c in [0,8) handles batch b = c//4 and head-group g = c%4
(q-heads 4g..4g+3, kv-head g).  Wq/Wk/Wv are column-sharded by head group,
Wo row-sharded; each core emits a partial output and the host sums the 4
partials per batch.

Per-core kernel (all matmuls free-dim 512 where possible, bf16 inputs with
fp32 PSUM accumulation):
  - qT/kT computed in [d, s] layout directly (weights pre-transposed on
    host); RoPE applied in rotate-half form (weight rows pre-permuted
    evens-then-odds on host) via DVE ops on [64, 512] tiles.
  - scores computed TRANSPOSED, sT[k, q] = kT.T-tile @ qT, so the PV matmul
    consumes exp(sT) directly with no on-chip transposes.
  - softmax without max subtraction (scores ~N(0, 0.8); exp is safe in f32),
    denominator accumulated in f32 SBUF and reduced with a ones-matmul,
    normalization broadcast via a K=1 matmul + DVE multiply.
"""

import numpy as np
import ml_dtypes

try:
    import concourse  # noqa: F401
except ImportError:  # pragma: no cover - path fallback
    import sys

    for _p in ("/root/.axon_site/_ro/trn_rl_repo", "/opt/trn_rl_repo"):
        if _p not in sys.path:
            sys.path.append(_p)

from contextlib import ExitStack

import concourse.bass as bass
import concourse.tile as tile
from concourse import bacc, mybir
from concourse.bass_utils import run_bass_kernel_spmd

F32 = mybir.dt.float32
BF16 = mybir.dt.bfloat16

B = 2
S = 2048
HID = 2048
HEADS = 16
KV_HEADS = 4
HD = 128
HALF = HD // 2
QH = HEADS // KV_HEADS  # q heads per core (4)
LO = QH * HD  # local q/o width (512)
N_CORES = 8

NEG = -1.0e5  # additive causal mask value (exp -> exactly 0 in f32)


def _emit(ctx: ExitStack, tc: "tile.TileContext", aps: dict, s_len: int):
    nc = tc.nc
    IT = HID // 128  # contraction tiles (16)
    SC = s_len // 512  # s-chunks of 512
    KBT = s_len // 128  # 128-wide k blocks
    QBT = s_len // 512  # 512-wide q blocks

    xT, wqT, wkT, wvT, woT = aps["xT"], aps["wqT"], aps["wkT"], aps["wvT"], aps["woT"]
    cosq, sinq, cosk, sink = aps["cosq"], aps["sinq"], aps["cosk"], aps["sink"]
    mtri, outp = aps["mtri"], aps["outp"]

    # ---- pools ----
    xpool = ctx.enter_context(tc.tile_pool(name="xpool", bufs=2))
    spsum = ctx.enter_context(tc.tile_pool(name="spsum", bufs=3, space="PSUM"))
    ypsum = ctx.enter_context(tc.tile_pool(name="ypsum", bufs=3, space="PSUM"))
    lpsum = ctx.enter_context(tc.tile_pool(name="lpsum", bufs=2, space="PSUM"))
    ptpool = ctx.enter_context(tc.tile_pool(name="ptpool", bufs=4))
    ropet = ctx.enter_context(tc.tile_pool(name="ropet", bufs=4))
    bcpool = ctx.enter_context(tc.tile_pool(name="bcpool", bufs=2))
    invpool = ctx.enter_context(tc.tile_pool(name="invpool", bufs=2))
    outpool = ctx.enter_context(tc.tile_pool(name="outpool", bufs=3))

    # ---- persistent SBUF tensors ----
    def single(shape, dtype, name):
        t, free = tc.tile(shape, dtype, name=name)
        ctx.callback(free)
        return t

    wq_sb = single([128, IT, LO], BF16, "wq_sb")
    wk_sb = single([128, IT, HD], BF16, "wk_sb")
    wv_sb = single([128, IT, HD], BF16, "wv_sb")
    wo_sb = single([128, QH, HID], BF16, "wo_sb")
    cq_sb = single([128, s_len], F32, "cq_sb")  # [cos; cos] (q-scaled)
    sq_sb = single([128, s_len], F32, "sq_sb")  # [-sin; sin] (q-scaled)
    ck_sb = single([128, s_len], F32, "ck_sb")
    sk_sb = single([128, s_len], F32, "sk_sb")
    mtri_sb = single([128, 128], F32, "mtri_sb")
    qT_sb = single([128, QH, s_len], BF16, "qT_sb")
    kT_sb = single([128, s_len], BF16, "kT_sb")
    v_sb = single([128, KBT, HD], BF16, "v_sb")
    yT_sb = single([128, QH, s_len], BF16, "yT_sb")
    ones_col = single([128, 1], BF16, "ones_col")

    nc.vector.memset(ones_col, 1.0)

    # Weights on the sync DMA queue, x chunks on the gpsimd queue (parallel
    # rings) so the first matmuls start ~6us in.  wo is deferred until after
    # phase 1 so it doesn't delay startup.
    # All inputs are host-packed into the exact SBUF layout (contiguous per
    # partition), so every DMA moves maximal contiguous lines.  Spread loads
    # over independent DMA rings: weights on sync, tables on scalar, x chunks
    # on gpsimd.
    for i4 in range(IT // 4):
        nc.sync.dma_start(
            out=wq_sb[:, i4 * 4 : (i4 + 1) * 4, :],
            in_=wqT[:, i4 * 4 : (i4 + 1) * 4, :],
        )
    nc.scalar.dma_start(out=cq_sb, in_=cosq)
    nc.scalar.dma_start(out=sq_sb, in_=sinq)
    nc.scalar.dma_start(out=ck_sb, in_=cosk)
    nc.scalar.dma_start(out=sk_sb, in_=sink)
    nc.scalar.dma_start(out=mtri_sb, in_=mtri)
    nc.scalar.dma_start(out=wk_sb, in_=wkT)
    nc.scalar.dma_start(out=wv_sb, in_=wvT)

    def rope(ps, out_full, cos_sb, sin_sb, sc):
        """out = RoPE(ps) in rotate-half layout; ps is a [128, 512] psum tile
        whose partitions are [evens(64); odds(64)] of one head.
        out = ps * C2 + swap_halves(ps) * S2, C2 = [cos;cos], S2 = [-sin;sin].
        """
        cs = cos_sb[:, sc * 512 : (sc + 1) * 512]
        sn = sin_sb[:, sc * 512 : (sc + 1) * 512]
        m1 = ropet.tile([128, 512], F32, tag="m1", name="m1")
        m2 = ropet.tile([128, 512], F32, tag="m2", name="m2")
        nc.vector.tensor_mul(m1, ps, cs)
        nc.vector.tensor_mul(m2[0:HALF, :], ps[HALF:128, :], sn[0:HALF, :])
        nc.vector.tensor_mul(m2[HALF:128, :], ps[0:HALF, :], sn[HALF:128, :])
        nc.vector.tensor_add(out_full, m1, m2)

    # ---------------- phase 1: Q/K/V projections + RoPE ----------------
    for sc in range(SC):
        xs = xpool.tile([128, IT, 512], BF16, tag="xs", name="xs")
        for i4 in range(IT // 4):
            nc.gpsimd.dma_start(
                out=xs[:, i4 * 4 : (i4 + 1) * 4, :],
                in_=xT[sc, :, i4 * 4 : (i4 + 1) * 4, :],
            )

        sl = slice(sc * 512, (sc + 1) * 512)
        # Q: per head, accumulate over i-tiles -> [128(d), 512(s)] psum
        for h in range(QH):
            ps_q = spsum.tile([128, 512], F32, tag="ps", name="ps_q")
            for it in range(IT):
                nc.tensor.matmul(
                    ps_q,
                    wq_sb[:, it, h * HD : (h + 1) * HD],
                    xs[:, it, :],
                    start=(it == 0),
                    stop=(it == IT - 1),
                )
            rope(ps_q, qT_sb[:, h, sl], cq_sb, sq_sb, sc)

        # K: one kv head
        ps_k = spsum.tile([128, 512], F32, tag="ps", name="ps_k")
        for it in range(IT):
            nc.tensor.matmul(
                ps_k,
                wk_sb[:, it, :],
                xs[:, it, :],
                start=(it == 0),
                stop=(it == IT - 1),
            )
        rope(ps_k, kT_sb[:, sl], ck_sb, sk_sb, sc)

        # V: natural [s, d] layout, four 128-row s-tiles per chunk
        for sj in range(4):
            st = sc * 4 + sj
            ps_v = spsum.tile([128, 512], F32, tag="ps", name="ps_v")
            for it in range(IT):
                nc.tensor.matmul(
                    ps_v[:, 0:HD],
                    xs[:, it, sj * 128 : (sj + 1) * 128],
                    wv_sb[:, it, :],
                    start=(it == 0),
                    stop=(it == IT - 1),
                )
            nc.scalar.copy(v_sb[:, st, :], ps_v[:, 0:HD])

    # prefetch wo during the attention phase
    nc.sync.dma_start(out=wo_sb, in_=woT)

    # ---------------- phase 2: attention ----------------
    # Software pipelining: emit the scores matmul for block kb, and the rest
    # of the chain (mask/exp/accumulate/PV) for block kb-LOOKAHEAD, so the PE
    # isn't stalled on the ACT exp of the immediately preceding block.
    LOOKAHEAD = 2
    pending = []  # deferred per-(h,J) finalizers

    def flush_pending():
        while pending:
            pending.pop(0)()

    for h in range(QH):
        for J in range(QBT):
            nkb = 4 * J + 4
            qsl = slice(J * 512, (J + 1) * 512)

            ps_y = ypsum.tile([128, 512], F32, tag="ps_y", name="ps_y")
            ps_l = lpsum.tile([1, 512], F32, tag="ps_l", name="ps_l")

            rest_q = []

            def emit_rest(kb, ps_s, h=h, J=J, ps_y=ps_y, ps_l=ps_l):
                nkb_ = 4 * J + 4
                r = kb - 4 * J
                lo = r * 128 if r >= 0 else 0
                if r >= 0:
                    nc.vector.tensor_add(
                        ps_s[:, lo : lo + 128], ps_s[:, lo : lo + 128], mtri_sb
                    )
                pt = ptpool.tile([128, 512], BF16, tag="pt", name="pt")
                nc.scalar.activation(
                    pt[:, lo:512], ps_s[:, lo:512], mybir.ActivationFunctionType.Exp
                )
                # softmax denominator: accumulate column sums of pt on the PE
                nc.tensor.matmul(
                    ps_l[:, lo:512],
                    ones_col,
                    pt[:, lo:512],
                    start=(kb == 0),
                    stop=(kb == nkb_ - 1),
                )
                nc.tensor.matmul(
                    ps_y[:, lo:512],
                    v_sb[:, kb, :],
                    pt[:, lo:512],
                    start=(kb == 0),
                    stop=(kb == nkb_ - 1),
                )

            for kb in range(nkb):
                r = kb - 4 * J  # >=0 on diagonal blocks
                lo = r * 128 if r >= 0 else 0

                ps_s = spsum.tile([128, 512], F32, tag="ps", name="ps_s")
                nc.tensor.matmul(
                    ps_s[:, lo:512],
                    kT_sb[:, kb * 128 : (kb + 1) * 128],
                    qT_sb[:, h, J * 512 + lo : (J + 1) * 512],
                    start=True,
                    stop=True,
                )
                rest_q.append((kb, ps_s))
                if kb == 1:
                    flush_pending()
                if len(rest_q) > LOOKAHEAD:
                    emit_rest(*rest_q.pop(0))
            while rest_q:
                emit_rest(*rest_q.pop(0))

            def finalize(h=h, J=J, ps_y=ps_y, ps_l=ps_l, qsl=qsl):
                l_sb = invpool.tile([1, 512], F32, tag="l_sb", name="l_sb")
                nc.vector.tensor_copy(l_sb, ps_l)
                # broadcast l across partitions (GPSIMD), then 1/l on DVE
                lbc = bcpool.tile([128, 512], F32, tag="lbc", name="lbc")
                nc.gpsimd.partition_broadcast(lbc, l_sb)
                rinv = invpool.tile([128, 512], F32, tag="rinv", name="rinv")
                nc.vector.reciprocal_approx_fast(rinv, lbc)
                nc.vector.tensor_mul(yT_sb[:, h, qsl], ps_y, rinv)

            pending.append(finalize)

    flush_pending()

    # ---------------- phase 3: output projection ----------------
    for st in range(KBT):
        for ob in range(HID // 512):
            ps_o = ypsum.tile([128, 512], F32, tag="ps_y", name="ps_o")
            for h in range(QH):
                nc.tensor.matmul(
                    ps_o,
                    yT_sb[:, h, st * 128 : (st + 1) * 128],
                    wo_sb[:, h, ob * 512 : (ob + 1) * 512],
                    start=(h == 0),
                    stop=(h == QH - 1),
                )
            o_sb = outpool.tile([128, 512], F32, tag="o_sb", name="o_sb")
            if (st * (HID // 512) + ob) % 2 == 0:
                nc.scalar.copy(o_sb, ps_o)
            else:
                nc.vector.tensor_copy(o_sb, ps_o)
            nc.sync.dma_start(out=outp[st, ob], in_=o_sb)


def build_module(s_len: int = S):
    nc = bacc.Bacc(
        "TRN2", target_bir_lowering=False, debug=False, enable_asserts=False
    )
    IT = HID // 128
    SC = s_len // 512
    aps = {}
    aps["xT"] = nc.dram_tensor(
        "xT", [SC, 128, IT, 512], BF16, kind="ExternalInput"
    ).ap()
    aps["wqT"] = nc.dram_tensor("wqT", [128, IT, LO], BF16, kind="ExternalInput").ap()
    aps["wkT"] = nc.dram_tensor("wkT", [128, IT, HD], BF16, kind="ExternalInput").ap()
    aps["wvT"] = nc.dram_tensor("wvT", [128, IT, HD], BF16, kind="ExternalInput").ap()
    aps["woT"] = nc.dram_tensor("woT", [128, QH, HID], BF16, kind="ExternalInput").ap()
    aps["cosq"] = nc.dram_tensor("cosq", [128, s_len], F32, kind="ExternalInput").ap()
    aps["sinq"] = nc.dram_tensor("sinq", [128, s_len], F32, kind="ExternalInput").ap()
    aps["cosk"] = nc.dram_tensor("cosk", [128, s_len], F32, kind="ExternalInput").ap()
    aps["sink"] = nc.dram_tensor("sink", [128, s_len], F32, kind="ExternalInput").ap()
    aps["mtri"] = nc.dram_tensor("mtri", [128, 128], F32, kind="ExternalInput").ap()
    aps["outp"] = nc.dram_tensor(
        "outp", [s_len // 128, HID // 512, 128, 512], F32, kind="ExternalOutput"
    ).ap()

    with tile.TileContext(nc) as tc:
        with ExitStack() as ctx:
            _emit(ctx, tc, aps, s_len)
    nc.compile()
    return nc


_MODULE_CACHE: dict = {}


def _get_module(s_len: int = S):
    if s_len not in _MODULE_CACHE:
        _MODULE_CACHE[s_len] = build_module(s_len)
    return _MODULE_CACHE[s_len]


_PERM = np.concatenate([np.arange(0, HD, 2), np.arange(1, HD, 2)])  # evens|odds


def make_in_maps(x, cos, sin, Wq, Wk, Wv, Wo, s_len: int = S):
    """Build the 8 per-core input maps (host-side sharding + layout prep)."""
    x = np.asarray(x, dtype=np.float32)
    cos = np.asarray(cos, dtype=np.float32)
    sin = np.asarray(sin, dtype=np.float32)
    Wq = np.asarray(Wq, dtype=np.float32)
    Wk = np.asarray(Wk, dtype=np.float32)
    Wv = np.asarray(Wv, dtype=np.float32)
    Wo = np.asarray(Wo, dtype=np.float32)

    bf = ml_dtypes.bfloat16
    scale = 1.0 / np.sqrt(HD)

    cosT = np.ascontiguousarray(cos.T)  # [64, S]
    sinT = np.ascontiguousarray(sin.T)
    # rotate-half tables: out = ps*[c;c] + swap_halves(ps)*[-s;s]
    cos2 = np.vstack([cosT, cosT])  # [128, S]
    sin2 = np.vstack([-sinT, sinT])
    cosq = (cos2 * scale).astype(np.float32)
    sinq = (sin2 * scale).astype(np.float32)
    cosk = cos2.astype(np.float32)
    sink = sin2.astype(np.float32)

    kk, qq = np.meshgrid(np.arange(128), np.arange(128), indexing="ij")
    mtri = np.where(kk <= qq, 0.0, NEG).astype(np.float32)

    Wq4 = Wq.reshape(HEADS, HD, HID)
    Wk4 = Wk.reshape(KV_HEADS, HD, HID)
    Wv4 = Wv.reshape(KV_HEADS, HD, HID)

    IT = HID // 128
    SC = s_len // 512

    def pack_w(w_l):  # [O, HID] -> [128(p), IT, O]; i = it*128 + p
        return np.ascontiguousarray(
            w_l.T.reshape(IT, 128, w_l.shape[0]).transpose(1, 0, 2)
        ).astype(bf)

    in_maps = []
    xT_cache = {}
    for c in range(N_CORES):
        b, g = divmod(c, KV_HEADS)
        hs = [g * QH + i for i in range(QH)]
        wq_l = Wq4[hs][:, _PERM, :].reshape(LO, HID)  # [512, 2048]
        wk_l = Wk4[g][_PERM, :]  # [128, 2048]
        wv_l = Wv4[g]  # [128, 2048]
        jcols = np.concatenate([np.arange(h * HD, (h + 1) * HD) for h in hs])
        wo_l = Wo[:, jcols]  # [2048, 512]

        if b not in xT_cache:
            # [SC, 128(p), IT, 512(s)]; x[b][sc*512+s, it*128+p]
            xT_cache[b] = np.ascontiguousarray(
                x[b]
                .reshape(SC, 512, IT, 128)
                .transpose(0, 3, 2, 1)
            ).astype(bf)

        in_maps.append(
            {
                "xT": xT_cache[b],
                "wqT": pack_w(wq_l),
                "wkT": pack_w(wk_l),
                "wvT": pack_w(wv_l),
                # [128(p), QH(jt), HID(o)]; j = jt*128 + p
                "woT": np.ascontiguousarray(
                    wo_l.T.reshape(QH, 128, HID).transpose(1, 0, 2)
                ).astype(bf),
                "cosq": cosq,
                "sinq": sinq,
                "cosk": cosk,
                "sink": sink,
                "mtri": mtri,
            }
        )
    return in_maps


def combine_outputs(results):
    out = np.zeros((B, S, HID), dtype=np.float32)
    for c in range(N_CORES):
        b = c // KV_HEADS
        # outp is [S//128, HID//512, 128, 512] device-layout
        part = results[c]["outp"].transpose(0, 2, 1, 3).reshape(S, HID)
        out[b] += part
    return out


def kernel(x, cos, sin, Wq, Wk, Wv, Wo):
    nc = _get_module(S)
    in_maps = make_in_maps(x, cos, sin, Wq, Wk, Wv, Wo, S)
    res = run_bass_kernel_spmd(nc, in_maps, core_ids=list(range(N_CORES)))
    return combine_outputs(res.results)


def run_traced(x, cos, sin, Wq, Wk, Wv, Wo, **trace_kwargs):
    """Like kernel() but with NTFF tracing; returns (output, BassKernelResults)."""
    nc = _get_module(S)
    in_maps = make_in_maps(x, cos, sin, Wq, Wk, Wv, Wo, S)
    res = run_bass_kernel_spmd(
        nc, in_maps, core_ids=list(range(N_CORES)), trace=True, **trace_kwargs
    )
    return combine_outputs(res.results), res
